# revision 19
# baseline (speedup 1.0000x reference)
"""Two-layer GAT (8-head 2->128, then 1-head 128->4 + log_softmax) on 8 TRN2 cores.

Strategy: destination-node sharding (per spec hint). Nodes are degree-sorted
and packed into 128-row tiles (round-robin across cores so all cores share one
compile-time slot schedule). Edges live in an ELL layout [dst partition, slot].

Layer 1 needs only the raw input x at each edge's source, so the per-edge
source values (x0, x1, +valid mask) are materialized host-side into the
edge-slot layout (same index-driven input sharding as the dst-side xd table)
and streamed sequentially - no device-side gather. a_src1 = x @ As is computed
on-device per edge from the 2 x values. Layer 1 aggregation uses the rank-2
structure of h1 = x @ W1: only sums of alpha*x (2 cols per head) are reduced,
then expanded through W1 per destination tile with PE matmuls.

Layer 2 depends on layer-1 output (h3 = relu(out1+b1) @ W2), so per-edge
source rows [h3(4) | a_src2] are gathered on-device from an allgathered table
with [P,1]-offset INDIRECT1D DMAs (this toolchain/HW supports exactly one
offset per partition per instruction; GPSIMD emits descriptors at ~8ns each,
making this phase the runtime floor). Segment softmax over destinations is
free-axis reductions; a_dst terms are per-partition scalars; max-subtraction
is skipped (value ranges keep exp well inside f32). LeakyReLU runs on the
vector engine so the scalar engine only ever loads the Exp table once.
"""

import os
import numpy as np
from contextlib import ExitStack

import concourse.bass as bass
import concourse.bacc as bacc
import concourse.tile as tile
from concourse import mybir
from concourse.bass import AP, IndirectOffsetOnAxis
from concourse.bass_utils import run_bass_kernel_spmd

P = 128
NCORE = 8
NEG = 0.2
EPS = 1e-16
NEGINF = -1.0e30
F32 = mybir.dt.float32
I32 = mybir.dt.int32

# consts column map
AS0X, AS1X, AD0, AD1 = 0, 16, 32, 40
W1BLK, W2EXT, B2, B1, IDENT = 48, 176, 182, 186, 187
CW = 320


def _v(t_ap: AP, off: int, dims) -> AP:
    """View with t_ap's partition dim and custom free dims [[step,count],...]."""
    return AP(t_ap.tensor, t_ap.offset + off, [list(t_ap.ap[0])] + [list(d) for d in dims])


def _dv(handle, off: int, dims) -> AP:
    """DRAM view with custom dims."""
    base = handle[:]
    return AP(base.tensor, off, [list(d) for d in dims])


def _plan(src: np.ndarray, dst: np.ndarray, N: int):
    """Host-side index preprocessing: degree sort, tiling, ELL layout."""
    E = src.shape[0]
    deg = np.bincount(dst, minlength=N).astype(np.int64)
    T = int(np.ceil(N / (P * NCORE)))          # local tiles per core
    NT = T * NCORE
    N_pad = NT * P
    order = np.concatenate([np.argsort(-deg, kind="stable"), np.arange(N, N_pad)])
    deg_pad = np.concatenate([deg, np.zeros(N_pad - N, np.int64)])
    odeg = deg_pad[order]
    tile_max = odeg.reshape(NT, P).max(axis=1)           # [NT] global tiles
    D_i = np.maximum(tile_max.reshape(T, NCORE).max(axis=1), 1)  # [T]

    # groups of consecutive local tiles sharing one slot width D
    GBUD = 192
    MAXGS = int(os.environ.get("GAT_GS", "8"))
    groups = []  # (i0, gs, D, off)
    off = 0
    i0 = 0
    while i0 < T:
        D = int(D_i[i0])
        gs = max(1, min(MAXGS, GBUD // D, T - i0))
        D = int(D_i[i0:i0 + gs].max())
        groups.append((i0, gs, D, off))
        off += gs * D
        i0 += gs
    S = off

    # column base per local tile
    colbase = np.zeros(T, np.int64)
    for (i0, gs, D, goff) in groups:
        for t in range(gs):
            colbase[i0 + t] = goff + t * D

    inv_order = np.empty(N_pad, np.int64)
    inv_order[order] = np.arange(N_pad)

    # pos2: row of node n in the (chunked) allgathered Z2 table (core-major)
    q = np.arange(N_pad)
    g = q // P
    pos_of_q = (g % NCORE) * (T * P) + (g // NCORE) * P + (q % P)
    pos2 = np.empty(N_pad, np.int64)
    pos2[order[q]] = pos_of_q

    # edges sorted by dst
    eorder = np.argsort(dst, kind="stable")
    dsts = dst[eorder]
    srcs = src[eorder]
    csr = np.zeros(N + 1, np.int64)
    csr[1:] = np.cumsum(deg)
    j = np.arange(E) - csr[dsts]              # rank within dst segment
    qe = inv_order[dsts]
    ge = qe // P
    de = qe % P
    ce = ge % NCORE
    ie = ge // NCORE
    cole = colbase[ie] + j

    srcidx1 = np.full((NCORE, P, S), N_pad, np.int32)
    srcidx2 = np.full((NCORE, P, S), N_pad, np.int32)
    srcidx1[ce, de, cole] = srcs.astype(np.int32)
    srcidx2[ce, de, cole] = pos2[srcs].astype(np.int32)

    # dst node ids per (core, partition, local tile)
    og = order.reshape(NT, P)                  # [g, d]
    dstid = np.empty((NCORE, P, T), np.int64)
    for c in range(NCORE):
        dstid[c] = og[c::NCORE].transpose(1, 0)  # [P, T]

    return dict(E=E, T=T, N_pad=N_pad, S=S, groups=groups, order=order,
                pos_of_q=pos_of_q, srcidx1=srcidx1, srcidx2=srcidx2, dstid=dstid,
                Dt=D_i.astype(np.int64))


def _consts(W1, att_src1, att_dst1, b1, W2, att_src2, att_dst2, b2):
    W1r = W1.reshape(2, 8, 16)
    As = np.einsum("khc,hc->kh", W1r, att_src1)    # [2, 8]
    Ad = np.einsum("khc,hc->kh", W1r, att_dst1)
    c = np.zeros((P, CW), np.float32)
    c[:, AS0X:AS0X + 8] = As[0]
    c[:, AS1X:AS1X + 8] = As[1]
    c[:, AD0:AD0 + 8] = Ad[0]
    c[:, AD1:AD1 + 8] = Ad[1]
    w1blk = np.zeros((16, 128), np.float32)
    for k in range(2):
        for h in range(8):
            w1blk[k * 8 + h, h * 16:(h + 1) * 16] = W1r[k, h]
    c[:16, W1BLK:W1BLK + 128] = w1blk
    c[:, W2EXT:W2EXT + 4] = W2
    c[:, W2EXT + 4] = W2 @ att_src2[0]
    c[:, W2EXT + 5] = W2 @ att_dst2[0]
    c[:, B2:B2 + 4] = b2
    c[:, B1] = b1
    c[:, IDENT:IDENT + 128] = np.eye(P, dtype=np.float32)
    dum = np.zeros((1, 24), np.float32)
    dum[0, 20] = NEGINF         # T2 dummy row cols 16:24 -> [h3=0, a_src2=-inf, ...]
    return c, dum


def _build(T, S, groups, N_pad, Dt):
    nc = bacc.Bacc("TRN2", target_bir_lowering=False)
    si2 = nc.declare_dram_parameter("si2", [P, S], I32, isOutput=False)
    xgin = nc.declare_dram_parameter("xg", [P, S * 3], F32, isOutput=False)
    xdin = nc.declare_dram_parameter("xd", [P, T * 2], F32, isOutput=False)
    cin = nc.declare_dram_parameter("consts", [P, CW], F32, isOutput=False)
    din = nc.declare_dram_parameter("dum", [1, 24], F32, isOutput=False)
    oext = nc.declare_dram_parameter("out", [T * P, 4], F32, isOutput=True)

    t2tab = nc.dram_tensor("t2tab", [N_pad + 1, 6], F32, addr_space="Shared")
    z2sh = nc.dram_tensor("z2sh", [T * P, 6], F32)

    ACT = mybir.ActivationFunctionType
    ALU = mybir.AluOpType

    with tile.TileContext(nc) as tc, ExitStack() as ctx:
        persist = ctx.enter_context(tc.tile_pool(name="persist", bufs=1))
        gp = ctx.enter_context(tc.tile_pool(name="gath", bufs=4))
        wk = ctx.enter_context(tc.tile_pool(name="work", bufs=3))
        w2 = ctx.enter_context(tc.tile_pool(name="work2", bufs=3))
        sm = ctx.enter_context(tc.tile_pool(name="small", bufs=4))
        pp = ctx.enter_context(tc.tile_pool(name="psA", bufs=2, space="PSUM"))
        pq = ctx.enter_context(tc.tile_pool(name="psB", bufs=3, space="PSUM"))

        csb = persist.tile([P, CW], F32)
        nc.sync.dma_start(out=csb[:], in_=cin[:])
        dsb = persist.tile([1, 24], F32)
        nc.sync.dma_start(out=dsb[:], in_=din[:])
        si2_sb = persist.tile([P, S], I32)
        nc.sync.dma_start(out=si2_sb[:], in_=si2[:])
        xg_sb = persist.tile([P, S * 3], F32)
        nc.sync.dma_start(out=xg_sb[:], in_=xgin[:])

        h3eS = persist.tile([P, T * 6], F32)
        adstE = persist.tile([P, T * 8], F32)
        nc.sync.dma_start(out=t2tab[N_pad:N_pad + 1, :], in_=dsb[0:1, 16:22])

        # a_dst per (partition, tile, head) from this core's dst-shard x rows
        xd = persist.tile([P, T * 2], F32)
        nc.sync.dma_start(out=xd[:], in_=xdin[:])
        ttd = persist.tile([P, T * 8], F32)
        nc.vector.tensor_tensor(
            out=adstE[:].rearrange("p (t h) -> p t h", h=8),
            in0=_v(xd[:], 0, [[2, T], [0, 8]]), in1=_v(csb[:], AD0, [[0, T], [1, 8]]),
            op=ALU.mult)
        nc.vector.tensor_tensor(
            out=ttd[:].rearrange("p (t h) -> p t h", h=8),
            in0=_v(xd[:], 1, [[2, T], [0, 8]]), in1=_v(csb[:], AD1, [[0, T], [1, 8]]),
            op=ALU.mult)
        nc.vector.tensor_tensor(out=adstE[:], in0=adstE[:], in1=ttd[:], op=ALU.add)

        tc.strict_bb_all_engine_barrier()

        # ---- layer 1 (host-pregathered x per edge slot) ----
        for (i0, gs, D, off) in groups:
            n8 = 8 * gs * D
            e = wk.tile([P, n8], F32, tag="e1")
            t1 = wk.tile([P, n8], F32, tag="t1")
            # e[h, tau, s] = x0*As0[h] + x1*As1[h] + adst[tau, h]
            nc.vector.tensor_tensor(
                out=_v(e[:], 0, [[gs * D, 8], [1, gs * D]]),
                in0=_v(xg_sb[:], off * 3, [[0, 8], [3, gs * D]]),
                in1=_v(csb[:], AS0X, [[1, 8], [0, gs * D]]),
                op=ALU.mult)
            nc.vector.tensor_tensor(
                out=_v(t1[:], 0, [[gs * D, 8], [1, gs * D]]),
                in0=_v(xg_sb[:], off * 3 + 1, [[0, 8], [3, gs * D]]),
                in1=_v(csb[:], AS1X, [[1, 8], [0, gs * D]]),
                op=ALU.mult)
            nc.vector.tensor_tensor(out=e[:], in0=e[:], in1=t1[:], op=ALU.add)
            nc.vector.tensor_tensor(
                out=_v(e[:], 0, [[gs * D, 8], [D, gs], [1, D]]),
                in0=_v(e[:], 0, [[gs * D, 8], [D, gs], [1, D]]),
                in1=_v(adstE[:], i0 * 8, [[1, 8], [8, gs], [0, D]]),
                op=ALU.add)
            # LeakyReLU on vector, Exp on scalar (keeps one act table loaded)
            nc.vector.tensor_scalar_mul(t1[:], e[:], NEG)
            nc.vector.tensor_tensor(out=e[:], in0=e[:], in1=t1[:], op=ALU.max)
            ex = wk.tile([P, n8], F32, tag="ex1")
            nc.scalar.activation(out=ex[:], in_=e[:], func=ACT.Exp)
            # zero padded slots (mask column: 1 valid, 0 pad)
            nc.vector.tensor_tensor(
                out=_v(ex[:], 0, [[gs * D, 8], [1, gs * D]]),
                in0=_v(ex[:], 0, [[gs * D, 8], [1, gs * D]]),
                in1=_v(xg_sb[:], off * 3 + 2, [[0, 8], [3, gs * D]]),
                op=ALU.mult)
            s = sm.tile([P, 8 * gs], F32, tag="s1")
            nc.vector.tensor_reduce(
                out=s[:], in_=ex[:].rearrange("p (a j) -> p a j", j=D),
                axis=mybir.AxisListType.X, op=ALU.add)
            rs = sm.tile([P, 8 * gs], F32, tag="rs1")
            nc.vector.tensor_scalar_add(rs[:], s[:], EPS)
            nc.vector.reciprocal(rs[:], rs[:])
            prod = wk.tile([P, 2 * n8], F32, tag="pr1")
            nc.vector.tensor_tensor(
                out=_v(prod[:], 0, [[n8, 2], [gs * D, 8], [1, gs * D]]),
                in0=_v(ex[:], 0, [[0, 2], [gs * D, 8], [1, gs * D]]),
                in1=_v(xg_sb[:], off * 3, [[1, 2], [0, 8], [3, gs * D]]),
                op=ALU.mult)
            G = sm.tile([P, 16 * gs], F32, tag="G1")
            nc.vector.tensor_reduce(
                out=G[:], in_=prod[:].rearrange("p (a j) -> p a j", j=D),
                axis=mybir.AxisListType.X, op=ALU.add)
            Gn = sm.tile([P, 16 * gs], F32, tag="Gn1")
            nc.vector.tensor_tensor(
                out=Gn[:].rearrange("p (k h t) -> p k h t", k=2, h=8),
                in0=G[:].rearrange("p (k h t) -> p k h t", k=2, h=8),
                in1=_v(rs[:], 0, [[0, 2], [gs, 8], [1, gs]]),
                op=ALU.mult)
            # per-tile transpose of [d, (k,h)] -> [16, d] at partition base 0,
            # concatenated along free dim so one matmul covers 4 tiles
            GnT = sm.tile([16, gs * 128], F32, tag="GnT")
            for t in range(gs):
                pt = pp.tile([P, P], F32, tag="pt")
                nc.tensor.transpose(
                    out=pt[0:16, :],
                    in_=_v(Gn[:], t, [[8 * gs, 2], [gs, 8]]),
                    identity=csb[:, IDENT:IDENT + 128])
                nc.scalar.copy(out=GnT[0:16, t * 128:(t + 1) * 128], in_=pt[0:16, :])
            for h0 in range(0, gs, 4):
                hn = min(4, gs - h0)
                o1p = pq.tile([P, 512], F32, tag="o1p")
                nc.tensor.matmul(
                    out=o1p[:, 0:hn * 128],
                    lhsT=csb[0:16, W1BLK:W1BLK + 128],
                    rhs=GnT[0:16, h0 * 128:(h0 + hn) * 128],
                    start=True, stop=True)
                h2T = wk.tile([P, 512], F32, tag="h2T")
                nc.scalar.activation(
                    out=h2T[:, 0:hn * 128], in_=o1p[:, 0:hn * 128],
                    func=ACT.Relu, bias=csb[:, B1:B1 + 1], scale=1.0)
                h3p = pq.tile([P, 32], F32, tag="h3p")
                for t in range(hn):
                    nc.tensor.matmul(
                        out=h3p[:, t * 8:t * 8 + 6],
                        lhsT=h2T[:, t * 128:(t + 1) * 128],
                        rhs=csb[:, W2EXT:W2EXT + 6],
                        start=True, stop=True)
                nc.vector.tensor_copy(
                    out=_v(h3eS[:], (i0 + h0) * 6, [[6, hn], [1, 6]]),
                    in_=_v(h3p[:], 0, [[8, hn], [1, 6]]))
                nc.sync.dma_start(
                    out=_dv(z2sh, (i0 + h0) * P * 6, [[6, P], [P * 6, hn], [1, 6]]),
                    in_=_v(h3eS[:], (i0 + h0) * 6, [[6, hn], [1, 6]]))

        # ---- share Z2 ----
        nc.gpsimd.collective_compute(
            "AllGather", ALU.bypass,
            replica_groups=[list(range(NCORE))],
            ins=[z2sh[:]], outs=[t2tab[0:N_pad, :]])

        # ---- layer 2 ----
        for (i0, gs, D, off) in groups:
            g2 = gp.tile([P, gs * D * 6], F32, tag="g2")
            for t in range(gs):
                dt = int(Dt[i0 + t])
                for c in range(dt):
                    sc = t * D + c
                    nc.gpsimd.indirect_dma_start(
                        out=g2[:, sc * 6:(sc + 1) * 6], out_offset=None,
                        in_=t2tab[:],
                        in_offset=IndirectOffsetOnAxis(
                            ap=si2_sb[:, off + sc:off + sc + 1], axis=0))
                if dt < D:
                    # fully-padded columns: a_src2 = -inf so exp -> 0
                    nc.vector.memset(g2[:, (t * D + dt) * 6:(t + 1) * D * 6], NEGINF)
            n1 = gs * D
            e2 = w2.tile([P, n1], F32, tag="e2")
            nc.vector.tensor_tensor(
                out=_v(e2[:], 0, [[D, gs], [1, D]]),
                in0=_v(g2[:], 4, [[6 * D, gs], [6, D]]),
                in1=_v(h3eS[:], i0 * 6 + 5, [[6, gs], [0, D]]),
                op=ALU.add)
            t2 = w2.tile([P, n1], F32, tag="t2")
            nc.vector.tensor_scalar_mul(t2[:], e2[:], NEG)
            nc.vector.tensor_tensor(out=e2[:], in0=e2[:], in1=t2[:], op=ALU.max)
            ex2 = w2.tile([P, n1], F32, tag="ex2")
            nc.scalar.activation(out=ex2[:], in_=e2[:], func=ACT.Exp)
            s2 = sm.tile([P, gs], F32, tag="s2")
            nc.vector.tensor_reduce(
                out=s2[:], in_=ex2[:].rearrange("p (t j) -> p t j", j=D),
                axis=mybir.AxisListType.X, op=ALU.add)
            rs2 = sm.tile([P, gs], F32, tag="rs2")
            nc.vector.tensor_scalar_add(rs2[:], s2[:], EPS)
            nc.vector.reciprocal(rs2[:], rs2[:])
            prod2 = w2.tile([P, 4 * n1], F32, tag="pr2")
            nc.vector.tensor_tensor(
                out=_v(prod2[:], 0, [[4 * D, gs], [D, 4], [1, D]]),
                in0=_v(ex2[:], 0, [[D, gs], [0, 4], [1, D]]),
                in1=_v(g2[:], 0, [[6 * D, gs], [1, 4], [6, D]]),
                op=ALU.mult)
            M2 = sm.tile([P, 4 * gs], F32, tag="M2")
            nc.vector.tensor_reduce(
                out=M2[:], in_=prod2[:].rearrange("p (a j) -> p a j", j=D),
                axis=mybir.AxisListType.X, op=ALU.add)
            o2 = sm.tile([P, 4 * gs], F32, tag="o2")
            nc.vector.tensor_tensor(
                out=o2[:].rearrange("p (t c) -> p t c", c=4),
                in0=M2[:].rearrange("p (t c) -> p t c", c=4),
                in1=_v(rs2[:], 0, [[1, gs], [0, 4]]),
                op=ALU.mult)
            nc.vector.tensor_tensor(
                out=o2[:].rearrange("p (t c) -> p t c", c=4),
                in0=o2[:].rearrange("p (t c) -> p t c", c=4),
                in1=_v(csb[:], B2, [[0, gs], [1, 4]]),
                op=ALU.add)
            # log_softmax over c
            mx = sm.tile([P, gs], F32, tag="mx")
            nc.vector.tensor_reduce(
                out=mx[:], in_=o2[:].rearrange("p (t c) -> p t c", c=4),
                axis=mybir.AxisListType.X, op=ALU.max)
            z = sm.tile([P, 4 * gs], F32, tag="z")
            nc.vector.tensor_tensor(
                out=z[:].rearrange("p (t c) -> p t c", c=4),
                in0=o2[:].rearrange("p (t c) -> p t c", c=4),
                in1=_v(mx[:], 0, [[1, gs], [0, 4]]),
                op=ALU.subtract)
            ez = sm.tile([P, 4 * gs], F32, tag="ez")
            nc.scalar.activation(out=ez[:], in_=z[:], func=ACT.Exp)
            se = sm.tile([P, gs], F32, tag="se")
            nc.vector.tensor_reduce(
                out=se[:], in_=ez[:].rearrange("p (t c) -> p t c", c=4),
                axis=mybir.AxisListType.X, op=ALU.add)
            lse = sm.tile([P, gs], F32, tag="lse")
            nc.scalar.activation(out=lse[:], in_=se[:], func=ACT.Ln)
            res = sm.tile([P, 4 * gs], F32, tag="res")
            nc.vector.tensor_tensor(
                out=res[:].rearrange("p (t c) -> p t c", c=4),
                in0=z[:].rearrange("p (t c) -> p t c", c=4),
                in1=_v(lse[:], 0, [[1, gs], [0, 4]]),
                op=ALU.subtract)
            nc.sync.dma_start(
                out=_dv(oext, i0 * P * 4, [[4, P], [P * 4, gs], [1, 4]]),
                in_=res[:].rearrange("p (t c) -> p t c", c=4))

    nc.compile()
    return nc


def kernel(**inputs) -> np.ndarray:
    x = np.asarray(inputs["x"], np.float32)
    edge_index = np.asarray(inputs["edge_index"])
    N = x.shape[0]
    src = edge_index[0].astype(np.int64)
    dst = edge_index[1].astype(np.int64)

    plan = _plan(src, dst, N)
    T, S, N_pad = plan["T"], plan["S"], plan["N_pad"]

    consts, dum = _consts(
        np.asarray(inputs["W1"], np.float32), np.asarray(inputs["att_src1"], np.float32),
        np.asarray(inputs["att_dst1"], np.float32), np.asarray(inputs["b1"], np.float32),
        np.asarray(inputs["W2"], np.float32), np.asarray(inputs["att_src2"], np.float32),
        np.asarray(inputs["att_dst2"], np.float32), np.asarray(inputs["b2"], np.float32))

    xpad = np.zeros((N_pad + 1, 2), np.float32)
    xpad[:N] = x

    nc = _build(T, S, plan["groups"], N_pad, plan["Dt"])

    in_maps = []
    for c in range(NCORE):
        si1 = plan["srcidx1"][c]                      # [P, S] node ids (N_pad = pad)
        xg = np.empty((P, S, 3), np.float32)
        xg[:, :, 0:2] = xpad[si1]
        xg[:, :, 2] = (si1 != N_pad).astype(np.float32)
        in_maps.append({
            "si2": plan["srcidx2"][c],
            "xg": xg.reshape(P, S * 3),
            "xd": xpad[plan["dstid"][c]].reshape(P, -1),
            "consts": consts,
            "dum": dum,
        })

    if os.environ.get("GAT_SIM", "0") == "1":
        from concourse.bass_interp import MultiCoreSim
        sim = MultiCoreSim(nc, NCORE)
        for c in range(NCORE):
            for k, v in in_maps[c].items():
                sim.cores[c].tensor(k)[:] = v
        sim.simulate()
        outs = [np.array(sim.cores[c].tensor("out")[:]) for c in range(NCORE)]
    else:
        trace = os.environ.get("GAT_TRACE", "0") == "1"
        res = run_bass_kernel_spmd(nc, in_maps, list(range(NCORE)), trace=trace)
        if trace:
            print(f"HW exec time: {res.exec_time_ns} ns")
        outs = [res.results[c]["out"] for c in range(NCORE)]

    big = np.concatenate(outs, axis=0)          # [NCORE*T*P, 4] core-major
    full = np.empty((N_pad, 4), np.float32)
    q = np.arange(N_pad)
    full[plan["order"][q]] = big[plan["pos_of_q"][q]]
    return full[:N]


# revision 20
# speedup vs baseline: 1.1730x; 1.1730x over previous
"""Two-layer GAT (8-head 2->128, then 1-head 128->4 + log_softmax) on 8 TRN2 cores.

Strategy: destination-node sharding (per spec hint). Nodes are degree-sorted
and packed into 128-row tiles (round-robin across cores so all cores share one
compile-time slot schedule). Edges live in an ELL layout [dst partition, slot].

Layer 1 needs only the raw input x at each edge's source, so the per-edge
source values (x0, x1, +valid mask) are materialized host-side into the
edge-slot layout (same index-driven input sharding as the dst-side xd table)
and streamed sequentially - no device-side gather. a_src1 = x @ As is computed
on-device per edge from the 2 x values. Layer 1 aggregation uses the rank-2
structure of h1 = x @ W1: only sums of alpha*x (2 cols per head) are reduced,
then expanded through W1 per destination tile with PE matmuls.

Layer 2 depends on layer-1 output (h3 = relu(out1+b1) @ W2), so per-edge
source rows [h3(4) | a_src2] are gathered on-device from an allgathered table
with [P,1]-offset INDIRECT1D DMAs (this toolchain/HW supports exactly one
offset per partition per instruction; GPSIMD emits descriptors at ~8ns each,
making this phase the runtime floor). Segment softmax over destinations is
free-axis reductions; a_dst terms are per-partition scalars; max-subtraction
is skipped (value ranges keep exp well inside f32). LeakyReLU runs on the
vector engine so the scalar engine only ever loads the Exp table once.
"""

import os
import numpy as np
from contextlib import ExitStack

import concourse.bass as bass
import concourse.bacc as bacc
import concourse.tile as tile
from concourse import mybir
from concourse.bass import AP, IndirectOffsetOnAxis
from concourse.bass_utils import run_bass_kernel_spmd

P = 128
NCORE = 8
NEG = 0.2
EPS = 1e-16
NEGINF = -1.0e30
F32 = mybir.dt.float32
I32 = mybir.dt.int32

# consts column map
AS0X, AS1X, AD0, AD1 = 0, 16, 32, 40
W1BLK, W2EXT, B2, B1, IDENT = 48, 176, 182, 186, 187
CW = 320


def _v(t_ap: AP, off: int, dims) -> AP:
    """View with t_ap's partition dim and custom free dims [[step,count],...]."""
    return AP(t_ap.tensor, t_ap.offset + off, [list(t_ap.ap[0])] + [list(d) for d in dims])


def _dv(handle, off: int, dims) -> AP:
    """DRAM view with custom dims."""
    base = handle[:]
    return AP(base.tensor, off, [list(d) for d in dims])


def _plan(src: np.ndarray, dst: np.ndarray, N: int):
    """Host-side index preprocessing: degree sort, tiling, ELL layout."""
    E = src.shape[0]
    deg = np.bincount(dst, minlength=N).astype(np.int64)
    T = int(np.ceil(N / (P * NCORE)))          # local tiles per core
    NT = T * NCORE
    N_pad = NT * P
    order = np.concatenate([np.argsort(-deg, kind="stable"), np.arange(N, N_pad)])
    deg_pad = np.concatenate([deg, np.zeros(N_pad - N, np.int64)])
    odeg = deg_pad[order]
    tile_max = odeg.reshape(NT, P).max(axis=1)           # [NT] global tiles
    D_i = np.maximum(tile_max.reshape(T, NCORE).max(axis=1), 1)  # [T]

    # groups of consecutive local tiles sharing one slot width D
    GBUD = 192
    MAXGS = int(os.environ.get("GAT_GS", "8"))
    groups = []  # (i0, gs, D, off)
    off = 0
    i0 = 0
    while i0 < T:
        D = int(D_i[i0])
        gs = max(1, min(MAXGS, GBUD // D, T - i0))
        D = int(D_i[i0:i0 + gs].max())
        groups.append((i0, gs, D, off))
        off += gs * D
        i0 += gs
    S = off

    # column base per local tile
    colbase = np.zeros(T, np.int64)
    for (i0, gs, D, goff) in groups:
        for t in range(gs):
            colbase[i0 + t] = goff + t * D

    inv_order = np.empty(N_pad, np.int64)
    inv_order[order] = np.arange(N_pad)

    # pos2: row of node n in the (chunked) allgathered Z2 table (core-major)
    q = np.arange(N_pad)
    g = q // P
    pos_of_q = (g % NCORE) * (T * P) + (g // NCORE) * P + (q % P)
    pos2 = np.empty(N_pad, np.int64)
    pos2[order[q]] = pos_of_q

    # edges sorted by dst
    eorder = np.argsort(dst, kind="stable")
    dsts = dst[eorder]
    srcs = src[eorder]
    csr = np.zeros(N + 1, np.int64)
    csr[1:] = np.cumsum(deg)
    j = np.arange(E) - csr[dsts]              # rank within dst segment
    qe = inv_order[dsts]
    ge = qe // P
    de = qe % P
    ce = ge % NCORE
    ie = ge // NCORE
    cole = colbase[ie] + j

    srcidx1 = np.full((NCORE, P, S), N_pad, np.int32)
    srcidx2 = np.full((NCORE, P, S), N_pad, np.int32)
    srcidx1[ce, de, cole] = srcs.astype(np.int32)
    srcidx2[ce, de, cole] = pos2[srcs].astype(np.int32)

    # dst node ids per (core, partition, local tile)
    og = order.reshape(NT, P)                  # [g, d]
    dstid = np.empty((NCORE, P, T), np.int64)
    for c in range(NCORE):
        dstid[c] = og[c::NCORE].transpose(1, 0)  # [P, T]

    return dict(E=E, T=T, N_pad=N_pad, S=S, groups=groups, order=order,
                pos_of_q=pos_of_q, srcidx1=srcidx1, srcidx2=srcidx2, dstid=dstid,
                Dt=D_i.astype(np.int64))


def _consts(W1, att_src1, att_dst1, b1, W2, att_src2, att_dst2, b2):
    W1r = W1.reshape(2, 8, 16)
    As = np.einsum("khc,hc->kh", W1r, att_src1)    # [2, 8]
    Ad = np.einsum("khc,hc->kh", W1r, att_dst1)
    c = np.zeros((P, CW), np.float32)
    c[:, AS0X:AS0X + 8] = As[0]
    c[:, AS1X:AS1X + 8] = As[1]
    c[:, AD0:AD0 + 8] = Ad[0]
    c[:, AD1:AD1 + 8] = Ad[1]
    w1blk = np.zeros((16, 128), np.float32)
    for k in range(2):
        for h in range(8):
            w1blk[k * 8 + h, h * 16:(h + 1) * 16] = W1r[k, h]
    c[:16, W1BLK:W1BLK + 128] = w1blk
    c[:, W2EXT:W2EXT + 4] = W2
    c[:, W2EXT + 4] = W2 @ att_src2[0]
    c[:, W2EXT + 5] = W2 @ att_dst2[0]
    c[:, B2:B2 + 4] = b2
    c[:, B1] = b1
    c[:, IDENT:IDENT + 128] = np.eye(P, dtype=np.float32)
    dum = np.zeros((1, 24), np.float32)
    dum[0, 20] = NEGINF         # T2 dummy row cols 16:24 -> [h3=0, a_src2=-inf, ...]
    return c, dum


def _build(T, S, groups, N_pad, Dt):
    nc = bacc.Bacc("TRN2", target_bir_lowering=False)
    si2 = nc.declare_dram_parameter("si2", [P, S], I32, isOutput=False)
    xgin = nc.declare_dram_parameter("xg", [P, S * 3], F32, isOutput=False)
    xdin = nc.declare_dram_parameter("xd", [P, T * 2], F32, isOutput=False)
    cin = nc.declare_dram_parameter("consts", [P, CW], F32, isOutput=False)
    din = nc.declare_dram_parameter("dum", [1, 24], F32, isOutput=False)
    oext = nc.declare_dram_parameter("out", [T * P, 4], F32, isOutput=True)

    t2tab = nc.dram_tensor("t2tab", [N_pad + 1, 6], F32, addr_space="Shared")
    z2sh = nc.dram_tensor("z2sh", [T * P, 6], F32)

    ACT = mybir.ActivationFunctionType
    ALU = mybir.AluOpType

    with tile.TileContext(nc) as tc, ExitStack() as ctx:
        persist = ctx.enter_context(tc.tile_pool(name="persist", bufs=1))
        gp = ctx.enter_context(tc.tile_pool(name="gath", bufs=4))
        wk = ctx.enter_context(tc.tile_pool(name="work", bufs=3))
        w2 = ctx.enter_context(tc.tile_pool(name="work2", bufs=3))
        sm = ctx.enter_context(tc.tile_pool(name="small", bufs=4))
        pp = ctx.enter_context(tc.tile_pool(name="psA", bufs=2, space="PSUM"))
        pq = ctx.enter_context(tc.tile_pool(name="psB", bufs=3, space="PSUM"))

        csb = persist.tile([P, CW], F32)
        nc.sync.dma_start(out=csb[:], in_=cin[:])
        dsb = persist.tile([1, 24], F32)
        nc.sync.dma_start(out=dsb[:], in_=din[:])
        si2_sb = persist.tile([P, S], I32)
        nc.sync.dma_start(out=si2_sb[:], in_=si2[:])
        xg_sb = persist.tile([P, S * 3], F32)
        nc.sync.dma_start(out=xg_sb[:], in_=xgin[:])

        h3eS = persist.tile([P, T * 6], F32)
        adstE = persist.tile([P, T * 8], F32)
        nc.sync.dma_start(out=t2tab[N_pad:N_pad + 1, :], in_=dsb[0:1, 16:22])

        # a_dst per (partition, tile, head) from this core's dst-shard x rows
        xd = persist.tile([P, T * 2], F32)
        nc.sync.dma_start(out=xd[:], in_=xdin[:])
        ttd = persist.tile([P, T * 8], F32)
        nc.vector.tensor_tensor(
            out=adstE[:].rearrange("p (t h) -> p t h", h=8),
            in0=_v(xd[:], 0, [[2, T], [0, 8]]), in1=_v(csb[:], AD0, [[0, T], [1, 8]]),
            op=ALU.mult)
        nc.vector.tensor_tensor(
            out=ttd[:].rearrange("p (t h) -> p t h", h=8),
            in0=_v(xd[:], 1, [[2, T], [0, 8]]), in1=_v(csb[:], AD1, [[0, T], [1, 8]]),
            op=ALU.mult)
        nc.vector.tensor_tensor(out=adstE[:], in0=adstE[:], in1=ttd[:], op=ALU.add)

        tc.strict_bb_all_engine_barrier()

        # ---- layer 1 (host-pregathered x per edge slot) ----
        for (i0, gs, D, off) in groups:
            n8 = 8 * gs * D
            e = wk.tile([P, n8], F32, tag="e1")
            t1 = wk.tile([P, n8], F32, tag="t1")
            # e[h, tau, s] = x0*As0[h] + x1*As1[h] + adst[tau, h]
            nc.vector.tensor_tensor(
                out=_v(e[:], 0, [[gs * D, 8], [1, gs * D]]),
                in0=_v(xg_sb[:], off * 3, [[0, 8], [3, gs * D]]),
                in1=_v(csb[:], AS0X, [[1, 8], [0, gs * D]]),
                op=ALU.mult)
            nc.vector.tensor_tensor(
                out=_v(t1[:], 0, [[gs * D, 8], [1, gs * D]]),
                in0=_v(xg_sb[:], off * 3 + 1, [[0, 8], [3, gs * D]]),
                in1=_v(csb[:], AS1X, [[1, 8], [0, gs * D]]),
                op=ALU.mult)
            nc.vector.tensor_tensor(out=e[:], in0=e[:], in1=t1[:], op=ALU.add)
            nc.vector.tensor_tensor(
                out=_v(e[:], 0, [[gs * D, 8], [D, gs], [1, D]]),
                in0=_v(e[:], 0, [[gs * D, 8], [D, gs], [1, D]]),
                in1=_v(adstE[:], i0 * 8, [[1, 8], [8, gs], [0, D]]),
                op=ALU.add)
            # LeakyReLU on vector, Exp on scalar (keeps one act table loaded)
            nc.vector.tensor_scalar_mul(t1[:], e[:], NEG)
            nc.vector.tensor_tensor(out=e[:], in0=e[:], in1=t1[:], op=ALU.max)
            ex = wk.tile([P, n8], F32, tag="ex1")
            nc.scalar.activation(out=ex[:], in_=e[:], func=ACT.Exp)
            # zero padded slots (mask column: 1 valid, 0 pad)
            nc.vector.tensor_tensor(
                out=_v(ex[:], 0, [[gs * D, 8], [1, gs * D]]),
                in0=_v(ex[:], 0, [[gs * D, 8], [1, gs * D]]),
                in1=_v(xg_sb[:], off * 3 + 2, [[0, 8], [3, gs * D]]),
                op=ALU.mult)
            s = sm.tile([P, 8 * gs], F32, tag="s1")
            nc.vector.tensor_reduce(
                out=s[:], in_=ex[:].rearrange("p (a j) -> p a j", j=D),
                axis=mybir.AxisListType.X, op=ALU.add)
            rs = sm.tile([P, 8 * gs], F32, tag="rs1")
            nc.vector.tensor_scalar_add(rs[:], s[:], EPS)
            nc.vector.reciprocal(rs[:], rs[:])
            prod = wk.tile([P, 2 * n8], F32, tag="pr1")
            nc.vector.tensor_tensor(
                out=_v(prod[:], 0, [[n8, 2], [gs * D, 8], [1, gs * D]]),
                in0=_v(ex[:], 0, [[0, 2], [gs * D, 8], [1, gs * D]]),
                in1=_v(xg_sb[:], off * 3, [[1, 2], [0, 8], [3, gs * D]]),
                op=ALU.mult)
            G = sm.tile([P, 16 * gs], F32, tag="G1")
            nc.vector.tensor_reduce(
                out=G[:], in_=prod[:].rearrange("p (a j) -> p a j", j=D),
                axis=mybir.AxisListType.X, op=ALU.add)
            Gn = sm.tile([P, 16 * gs], F32, tag="Gn1")
            nc.vector.tensor_tensor(
                out=Gn[:].rearrange("p (k h t) -> p k h t", k=2, h=8),
                in0=G[:].rearrange("p (k h t) -> p k h t", k=2, h=8),
                in1=_v(rs[:], 0, [[0, 2], [gs, 8], [1, gs]]),
                op=ALU.mult)
            # per-tile transpose of [d, (k,h)] -> [16, d] at partition base 0,
            # concatenated along free dim so one matmul covers 4 tiles
            GnT = sm.tile([16, gs * 128], F32, tag="GnT")
            for t in range(gs):
                pt = pp.tile([P, P], F32, tag="pt")
                nc.tensor.transpose(
                    out=pt[0:16, :],
                    in_=_v(Gn[:], t, [[8 * gs, 2], [gs, 8]]),
                    identity=csb[:, IDENT:IDENT + 128])
                nc.scalar.copy(out=GnT[0:16, t * 128:(t + 1) * 128], in_=pt[0:16, :])
            for h0 in range(0, gs, 4):
                hn = min(4, gs - h0)
                o1p = pq.tile([P, 512], F32, tag="o1p")
                nc.tensor.matmul(
                    out=o1p[:, 0:hn * 128],
                    lhsT=csb[0:16, W1BLK:W1BLK + 128],
                    rhs=GnT[0:16, h0 * 128:(h0 + hn) * 128],
                    start=True, stop=True)
                h2T = wk.tile([P, 512], F32, tag="h2T")
                nc.scalar.activation(
                    out=h2T[:, 0:hn * 128], in_=o1p[:, 0:hn * 128],
                    func=ACT.Relu, bias=csb[:, B1:B1 + 1], scale=1.0)
                h3p = pq.tile([P, 32], F32, tag="h3p")
                for t in range(hn):
                    nc.tensor.matmul(
                        out=h3p[:, t * 8:t * 8 + 6],
                        lhsT=h2T[:, t * 128:(t + 1) * 128],
                        rhs=csb[:, W2EXT:W2EXT + 6],
                        start=True, stop=True)
                nc.vector.tensor_copy(
                    out=_v(h3eS[:], (i0 + h0) * 6, [[6, hn], [1, 6]]),
                    in_=_v(h3p[:], 0, [[8, hn], [1, 6]]))
                nc.sync.dma_start(
                    out=_dv(z2sh, (i0 + h0) * P * 6, [[6, P], [P * 6, hn], [1, 6]]),
                    in_=_v(h3eS[:], (i0 + h0) * 6, [[6, hn], [1, 6]]))

        # ---- share Z2 ----
        nc.gpsimd.collective_compute(
            "AllGather", ALU.bypass,
            replica_groups=[list(range(NCORE))],
            ins=[z2sh[:]], outs=[t2tab[0:N_pad, :]])
        tc.strict_bb_all_engine_barrier()

        # ---- layer 2 ----
        for (i0, gs, D, off) in groups:
            g2 = gp.tile([P, gs * D * 6], F32, tag="g2")
            for t in range(gs):
                dt = int(Dt[i0 + t])
                for c in range(dt):
                    sc = t * D + c
                    nc.gpsimd.indirect_dma_start(
                        out=g2[:, sc * 6:(sc + 1) * 6], out_offset=None,
                        in_=t2tab[:],
                        in_offset=IndirectOffsetOnAxis(
                            ap=si2_sb[:, off + sc:off + sc + 1], axis=0))
                if dt < D:
                    # fully-padded columns: a_src2 = -inf so exp -> 0
                    nc.vector.memset(g2[:, (t * D + dt) * 6:(t + 1) * D * 6], NEGINF)
            n1 = gs * D
            e2 = w2.tile([P, n1], F32, tag="e2")
            nc.vector.tensor_tensor(
                out=_v(e2[:], 0, [[D, gs], [1, D]]),
                in0=_v(g2[:], 4, [[6 * D, gs], [6, D]]),
                in1=_v(h3eS[:], i0 * 6 + 5, [[6, gs], [0, D]]),
                op=ALU.add)
            t2 = w2.tile([P, n1], F32, tag="t2")
            nc.vector.tensor_scalar_mul(t2[:], e2[:], NEG)
            nc.vector.tensor_tensor(out=e2[:], in0=e2[:], in1=t2[:], op=ALU.max)
            ex2 = w2.tile([P, n1], F32, tag="ex2")
            nc.scalar.activation(out=ex2[:], in_=e2[:], func=ACT.Exp)
            s2 = sm.tile([P, gs], F32, tag="s2")
            nc.vector.tensor_reduce(
                out=s2[:], in_=ex2[:].rearrange("p (t j) -> p t j", j=D),
                axis=mybir.AxisListType.X, op=ALU.add)
            rs2 = sm.tile([P, gs], F32, tag="rs2")
            nc.vector.tensor_scalar_add(rs2[:], s2[:], EPS)
            nc.vector.reciprocal(rs2[:], rs2[:])
            prod2 = w2.tile([P, 4 * n1], F32, tag="pr2")
            nc.vector.tensor_tensor(
                out=_v(prod2[:], 0, [[4 * D, gs], [D, 4], [1, D]]),
                in0=_v(ex2[:], 0, [[D, gs], [0, 4], [1, D]]),
                in1=_v(g2[:], 0, [[6 * D, gs], [1, 4], [6, D]]),
                op=ALU.mult)
            M2 = sm.tile([P, 4 * gs], F32, tag="M2")
            nc.vector.tensor_reduce(
                out=M2[:], in_=prod2[:].rearrange("p (a j) -> p a j", j=D),
                axis=mybir.AxisListType.X, op=ALU.add)
            o2 = sm.tile([P, 4 * gs], F32, tag="o2")
            nc.vector.tensor_tensor(
                out=o2[:].rearrange("p (t c) -> p t c", c=4),
                in0=M2[:].rearrange("p (t c) -> p t c", c=4),
                in1=_v(rs2[:], 0, [[1, gs], [0, 4]]),
                op=ALU.mult)
            nc.vector.tensor_tensor(
                out=o2[:].rearrange("p (t c) -> p t c", c=4),
                in0=o2[:].rearrange("p (t c) -> p t c", c=4),
                in1=_v(csb[:], B2, [[0, gs], [1, 4]]),
                op=ALU.add)
            # log_softmax over c
            mx = sm.tile([P, gs], F32, tag="mx")
            nc.vector.tensor_reduce(
                out=mx[:], in_=o2[:].rearrange("p (t c) -> p t c", c=4),
                axis=mybir.AxisListType.X, op=ALU.max)
            z = sm.tile([P, 4 * gs], F32, tag="z")
            nc.vector.tensor_tensor(
                out=z[:].rearrange("p (t c) -> p t c", c=4),
                in0=o2[:].rearrange("p (t c) -> p t c", c=4),
                in1=_v(mx[:], 0, [[1, gs], [0, 4]]),
                op=ALU.subtract)
            ez = sm.tile([P, 4 * gs], F32, tag="ez")
            nc.scalar.activation(out=ez[:], in_=z[:], func=ACT.Exp)
            se = sm.tile([P, gs], F32, tag="se")
            nc.vector.tensor_reduce(
                out=se[:], in_=ez[:].rearrange("p (t c) -> p t c", c=4),
                axis=mybir.AxisListType.X, op=ALU.add)
            lse = sm.tile([P, gs], F32, tag="lse")
            nc.scalar.activation(out=lse[:], in_=se[:], func=ACT.Ln)
            res = sm.tile([P, 4 * gs], F32, tag="res")
            nc.vector.tensor_tensor(
                out=res[:].rearrange("p (t c) -> p t c", c=4),
                in0=z[:].rearrange("p (t c) -> p t c", c=4),
                in1=_v(lse[:], 0, [[1, gs], [0, 4]]),
                op=ALU.subtract)
            nc.sync.dma_start(
                out=_dv(oext, i0 * P * 4, [[4, P], [P * 4, gs], [1, 4]]),
                in_=res[:].rearrange("p (t c) -> p t c", c=4))

    nc.compile()
    return nc


def kernel(**inputs) -> np.ndarray:
    x = np.asarray(inputs["x"], np.float32)
    edge_index = np.asarray(inputs["edge_index"])
    N = x.shape[0]
    src = edge_index[0].astype(np.int64)
    dst = edge_index[1].astype(np.int64)

    plan = _plan(src, dst, N)
    T, S, N_pad = plan["T"], plan["S"], plan["N_pad"]

    consts, dum = _consts(
        np.asarray(inputs["W1"], np.float32), np.asarray(inputs["att_src1"], np.float32),
        np.asarray(inputs["att_dst1"], np.float32), np.asarray(inputs["b1"], np.float32),
        np.asarray(inputs["W2"], np.float32), np.asarray(inputs["att_src2"], np.float32),
        np.asarray(inputs["att_dst2"], np.float32), np.asarray(inputs["b2"], np.float32))

    xpad = np.zeros((N_pad + 1, 2), np.float32)
    xpad[:N] = x

    nc = _build(T, S, plan["groups"], N_pad, plan["Dt"])

    in_maps = []
    for c in range(NCORE):
        si1 = plan["srcidx1"][c]                      # [P, S] node ids (N_pad = pad)
        xg = np.empty((P, S, 3), np.float32)
        xg[:, :, 0:2] = xpad[si1]
        xg[:, :, 2] = (si1 != N_pad).astype(np.float32)
        in_maps.append({
            "si2": plan["srcidx2"][c],
            "xg": xg.reshape(P, S * 3),
            "xd": xpad[plan["dstid"][c]].reshape(P, -1),
            "consts": consts,
            "dum": dum,
        })

    if os.environ.get("GAT_SIM", "0") == "1":
        from concourse.bass_interp import MultiCoreSim
        sim = MultiCoreSim(nc, NCORE)
        for c in range(NCORE):
            for k, v in in_maps[c].items():
                sim.cores[c].tensor(k)[:] = v
        sim.simulate()
        outs = [np.array(sim.cores[c].tensor("out")[:]) for c in range(NCORE)]
    else:
        trace = os.environ.get("GAT_TRACE", "0") == "1"
        res = run_bass_kernel_spmd(nc, in_maps, list(range(NCORE)), trace=trace)
        if trace:
            print(f"HW exec time: {res.exec_time_ns} ns")
        outs = [res.results[c]["out"] for c in range(NCORE)]

    big = np.concatenate(outs, axis=0)          # [NCORE*T*P, 4] core-major
    full = np.empty((N_pad, 4), np.float32)
    q = np.arange(N_pad)
    full[plan["order"][q]] = big[plan["pos_of_q"][q]]
    return full[:N]
